# revision 11
# baseline (speedup 1.0000x reference)
"""Bahdanau attention on 8 Trainium2 NeuronCores (Bass/Tile).

Data-parallel over the batch dim: each of the 8 cores gets 32 of the 256
batch rows; all weights are replicated. Per core:

  proj_h = dec @ W_h.T                  (B, H)     [tiny matmul]
  proj_s = X @ W_s.T                    (B, L, H)  [17 GFLOP matmul]
  scores = v . tanh(proj_h + proj_s)    (B, L)
  alpha  = softmax(scores, axis=-1)     (B, L)
  ctx    = alpha @ X                    (B, E)

Device layout: all TensorE-facing data is bf16 (f32 PSUM accumulation).
The host ships X twice — transposed `xt` (e on partitions) for proj_s and
natural `xn` (l on partitions) for ctx — so the device never transposes
the big tensor. proj_s is computed transposed (h on partitions, l on the
free dim), which lets the proj_h add ride the ACT bias operand and makes
the v-dot a K=128, M=1 matmul accumulated over the four h chunks.

The 32 batches run in 4 rounds of 8 (2 col-groups of 4): the M=1 v-dot
and ctx matmuls for 4 batches are packed into distinct PE column groups
via tile_position=(0, 32j) so they stream concurrently, and round
pipelining overlaps the ctx pass's xn DMA with the next round's proj
matmuls.
"""

from contextlib import ExitStack

import ml_dtypes
import numpy as np

import concourse.tile as tile
from concourse import bacc, mybir
from concourse.bass_utils import run_bass_kernel_spmd
from concourse.masks import make_identity

F32 = mybir.dt.float32
BF16 = mybir.dt.bfloat16
AF = mybir.ActivationFunctionType

N_CORES = 8
B_TOTAL, L, E, H = 256, 1024, 512, 512
B = B_TOTAL // N_CORES
RND = 8  # batches per round (softmax batch)
GRP = 4  # batches per PE col-group pack


def build(B=B, L=L, E=E, H=H, reps=1):
    """Build + finalize the per-core Bacc module (SPMD: same on all cores).

    reps>1 wraps the body in a device-side For_i loop — timing only.
    """
    EC, HC, LC, L2 = E // 128, H // 128, L // 128, L // 512
    NR = B // RND
    NG = RND // GRP

    nc = bacc.Bacc()

    xt_d = nc.dram_tensor("xt", [B, EC, 128, L], BF16, kind="ExternalInput")
    xn_d = nc.dram_tensor("xn", [B, LC, 128, E], BF16, kind="ExternalInput")
    wst_d = nc.dram_tensor("wst", [EC, 128, H], BF16, kind="ExternalInput")
    wht_d = nc.dram_tensor("wht", [HC, 128, H], BF16, kind="ExternalInput")
    dect_d = nc.dram_tensor("dect", [HC, 128, B], BF16, kind="ExternalInput")
    vt_d = nc.dram_tensor("vt", [HC, 128, 1], BF16, kind="ExternalInput")
    ctx_d = nc.dram_tensor("ctx", [B, E], F32, kind="ExternalOutput")
    alpha_d = nc.dram_tensor("alpha", [B, L], F32, kind="ExternalOutput")

    with tile.TileContext(nc) as tc, ExitStack() as ctx:
        singles = ctx.enter_context(tc.tile_pool(name="singles", bufs=1))
        xt_pool = ctx.enter_context(tc.tile_pool(name="xt", bufs=GRP + 2))
        xn_pool = ctx.enter_context(tc.tile_pool(name="xn", bufs=GRP + 2))
        tanh_pool = ctx.enter_context(tc.tile_pool(name="tanh", bufs=GRP + 2))
        stage_pool = ctx.enter_context(tc.tile_pool(name="stage", bufs=3))
        rnd_pool = ctx.enter_context(tc.tile_pool(name="rnd", bufs=2))
        pp = ctx.enter_context(tc.tile_pool(name="pp", bufs=2, space="PSUM"))
        ps_sc = ctx.enter_context(tc.tile_pool(name="ps_sc", bufs=3, space="PSUM"))
        ps_mi = ctx.enter_context(tc.tile_pool(name="ps_mi", bufs=1, space="PSUM"))

        # --- static setup: weights, identity, proj_h ---
        wst_sb = singles.tile([128, EC, H], BF16)
        nc.sync.dma_start(out=wst_sb[:], in_=wst_d[:].rearrange("c p h -> p c h"))
        wht_sb = singles.tile([128, HC, H], BF16)
        nc.sync.dma_start(out=wht_sb[:], in_=wht_d[:].rearrange("c p h -> p c h"))
        dect_sb = singles.tile([128, HC, B], BF16)
        nc.sync.dma_start(out=dect_sb[:], in_=dect_d[:].rearrange("c p b -> p c b"))
        v_sb = singles.tile([128, HC, 1], BF16)
        nc.sync.dma_start(out=v_sb[:], in_=vt_d[:].rearrange("c p o -> p c o"))
        ident = singles.tile([RND, RND], F32)
        make_identity(nc, ident[:])

        # proj_h.T in (h, b) layout: ph_sb[p, hc, b] = proj_h[b, hc*128+p]
        ph_sb = singles.tile([128, HC, B], F32)
        for hc in range(HC):
            ph_ps = ps_mi.tile([128, B], F32, tag="mi")
            for kc in range(HC):
                nc.tensor.matmul(
                    ph_ps[:],
                    wht_sb[:, kc, hc * 128 : (hc + 1) * 128],
                    dect_sb[:, kc, :],
                    start=(kc == 0),
                    stop=(kc == HC - 1),
                )
            nc.vector.tensor_copy(ph_sb[:, hc, :], ph_ps[:])

        def emit_pass1(r):
            scores_all = rnd_pool.tile([RND, L], F32, tag="scores_all")

            # ---- pass 1: scores for the round's RND batches ----
            for g in range(NG):
                b0 = r * RND + g * GRP
                xt_ts = []
                for j in range(GRP):
                    xt_t = xt_pool.tile(
                        [128, EC, L], BF16, tag="xt", name=f"xt{b0}_{j}"
                    )
                    nc.sync.dma_start(
                        out=xt_t[:], in_=xt_d[b0 + j].rearrange("c p l -> p c l")
                    )
                    xt_ts.append(xt_t)

                sc_ps = [
                    ps_sc.tile([128, 512], F32, tag="sc", name=f"sc{b0}_{i}")
                    for i in range(L2)
                ]
                for l2 in range(L2):
                    nc.vector.memset(sc_ps[l2][:], 0.0)
                for hc in range(HC):
                    tanh_ts = []
                    for j in range(GRP):
                        proj = pp.tile([128, L], F32, tag="proj", name=f"pj{b0}_{j}")
                        for l2 in range(L2):
                            for ec in range(EC):
                                nc.tensor.matmul(
                                    proj[:, l2 * 512 : (l2 + 1) * 512],
                                    wst_sb[:, ec, hc * 128 : (hc + 1) * 128],
                                    xt_ts[j][:, ec, l2 * 512 : (l2 + 1) * 512],
                                    start=(ec == 0),
                                    stop=(ec == EC - 1),
                                )
                        tanh_t = tanh_pool.tile(
                            [128, L], BF16, tag="tanh", name=f"th{b0}_{j}"
                        )
                        nc.scalar.activation(
                            tanh_t[:], proj[:], AF.Tanh,
                            bias=ph_sb[:, hc, b0 + j : b0 + j + 1],
                        )
                        tanh_ts.append(tanh_t)
                    # 4 batches' v-dots packed into distinct PE col groups
                    for l2 in range(L2):
                        for j in range(GRP):
                            nc.tensor.matmul(
                                sc_ps[l2][32 * j : 32 * j + 1, :],
                                v_sb[:, hc, :],
                                tanh_ts[j][:, l2 * 512 : (l2 + 1) * 512],
                                start=(hc == 0),
                                stop=(hc == HC - 1),
                                tile_position=(0, 32 * j),
                                skip_group_check=True,
                            )
                sc_sb = stage_pool.tile([128, L], F32, tag="sc_sb")
                for l2 in range(L2):
                    nc.vector.tensor_copy(
                        sc_sb[:, l2 * 512 : (l2 + 1) * 512], sc_ps[l2][:]
                    )
                nc.sync.dma_start(
                    out=scores_all[g * GRP : (g + 1) * GRP, :],
                    in_=sc_sb[:].rearrange("(j q) l -> j q l", j=GRP)[:, 0, :],
                )

            return scores_all

        def emit_softmax(r, scores_all):
            # ---- softmax over l (RND rows on partitions) ----
            negmax = rnd_pool.tile([RND, 1], F32, tag="negmax")
            nc.vector.tensor_reduce(
                negmax[:], scores_all[:], axis=mybir.AxisListType.X,
                op=mybir.AluOpType.max, negate=True,
            )
            alpha_sb = rnd_pool.tile([RND, L], F32, tag="alpha")
            denom = rnd_pool.tile([RND, 1], F32, tag="denom")
            nc.scalar.activation(
                alpha_sb[:], scores_all[:], AF.Exp,
                bias=negmax[:], scale=1.0, accum_out=denom[:],
            )
            rden = rnd_pool.tile([RND, 1], F32, tag="rden")
            nc.vector.reciprocal(rden[:], denom[:])
            nc.vector.tensor_scalar_mul(alpha_sb[:], in0=alpha_sb[:], scalar1=rden[:])
            nc.sync.dma_start(out=alpha_d[r * RND : (r + 1) * RND, :], in_=alpha_sb[:])

            return alpha_sb

        def emit_ctx(r, alpha_sb):
            # alpha.T in bf16 for the ctx matmul: alphaT[p, lc, b_local]
            alphaT = rnd_pool.tile([128, LC, RND], BF16, tag="alphaT")
            for lc in range(LC):
                tp = ps_mi.tile([128, RND], F32, tag="mi")
                nc.tensor.transpose(
                    tp[:], alpha_sb[:, lc * 128 : (lc + 1) * 128], ident[:]
                )
                nc.vector.tensor_copy(alphaT[:, lc, :], tp[:])

            # ---- pass 2: ctx = alpha @ X (4 batches per PE col-group pack) ----
            for g in range(NG):
                b0 = r * RND + g * GRP
                xn_ts = []
                for j in range(GRP):
                    xn_t = xn_pool.tile(
                        [128, LC, E], BF16, tag="xn", name=f"xn{b0}_{j}"
                    )
                    nc.sync.dma_start(
                        out=xn_t[:], in_=xn_d[b0 + j].rearrange("c p e -> p c e")
                    )
                    xn_ts.append(xn_t)
                ctx_ps = ps_mi.tile([128, E], F32, tag="mi", name=f"cp{b0}")
                nc.vector.memset(ctx_ps[:], 0.0)
                for lc in range(LC):
                    for j in range(GRP):
                        nc.tensor.matmul(
                            ctx_ps[32 * j : 32 * j + 1, :],
                            alphaT[:, lc, g * GRP + j : g * GRP + j + 1],
                            xn_ts[j][:, lc, :],
                            start=(lc == 0),
                            stop=(lc == LC - 1),
                            tile_position=(0, 32 * j),
                            skip_group_check=True,
                        )
                ctx_sb = stage_pool.tile([128, E], F32, tag="ctx_sb")
                nc.vector.tensor_copy(ctx_sb[:], ctx_ps[:])
                nc.sync.dma_start(
                    out=ctx_d[b0 : b0 + GRP, :],
                    in_=ctx_sb[:].rearrange("(j q) e -> j q e", j=GRP)[:, 0, :],
                )

        def emit_body():
            pend = None
            for r in range(B // RND):
                if pend is not None:
                    al = emit_softmax(pend[0], pend[1])
                    pend = (pend[0], pend[1], al)
                sc = emit_pass1(r)
                if pend is not None:
                    emit_ctx(pend[0], pend[2])
                pend = (r, sc)
            al = emit_softmax(pend[0], pend[1])
            emit_ctx(pend[0], al)

        if reps == 1:
            emit_body()
        else:
            with tc.For_i(0, reps):
                emit_body()

    nc.finalize()
    return nc


def host_prep(decoder_hidden, encoder_outputs, W_h, W_s, v, n_cores=N_CORES):
    """Full f32 inputs -> per-core input dicts in the device bf16 layouts."""
    bf16 = ml_dtypes.bfloat16
    BT, L_, E_ = encoder_outputs.shape
    H_ = W_h.shape[0]
    Bc = BT // n_cores

    x16 = np.ascontiguousarray(encoder_outputs).astype(bf16)
    # xt[b, ec, p, l] = X[b, l, ec*128+p]
    xt = np.ascontiguousarray(x16.reshape(BT, L_, E_ // 128, 128).transpose(0, 2, 3, 1))
    # xn[b, lc, p, e] = X[b, lc*128+p, e]  (pure view)
    xn = x16.reshape(BT, L_ // 128, 128, E_)
    # wst[ec, p, h] = W_s[h, ec*128+p]
    wst = np.ascontiguousarray(W_s.astype(bf16).reshape(H_, E_ // 128, 128).transpose(1, 2, 0))
    wht = np.ascontiguousarray(W_h.astype(bf16).reshape(H_, H_ // 128, 128).transpose(1, 2, 0))
    # dect[kc, p, b] = dec[b, kc*128+p]
    dect_full = np.ascontiguousarray(
        decoder_hidden.astype(bf16).reshape(BT, H_ // 128, 128).transpose(1, 2, 0)
    )
    vt = np.ascontiguousarray(v.astype(bf16).reshape(H_ // 128, 128, 1))

    in_maps = []
    for c in range(n_cores):
        sl = slice(c * Bc, (c + 1) * Bc)
        in_maps.append(
            {
                "xt": xt[sl],
                "xn": xn[sl],
                "wst": wst,
                "wht": wht,
                "dect": np.ascontiguousarray(dect_full[:, :, sl]),
                "vt": vt,
            }
        )
    return in_maps


_NC_CACHE = {}


def _get_nc(reps=1):
    if reps not in _NC_CACHE:
        _NC_CACHE[reps] = build(reps=reps)
    return _NC_CACHE[reps]


def kernel(decoder_hidden, encoder_outputs, W_h, W_s, v):
    decoder_hidden = np.asarray(decoder_hidden, dtype=np.float32)
    encoder_outputs = np.asarray(encoder_outputs, dtype=np.float32)
    W_h = np.asarray(W_h, dtype=np.float32)
    W_s = np.asarray(W_s, dtype=np.float32)
    v = np.asarray(v, dtype=np.float32)
    assert decoder_hidden.shape == (B_TOTAL, H)
    assert encoder_outputs.shape == (B_TOTAL, L, E)

    in_maps = host_prep(decoder_hidden, encoder_outputs, W_h, W_s, v)
    nc = _get_nc(reps=1)
    res = run_bass_kernel_spmd(nc, in_maps, core_ids=list(range(N_CORES)))
    context = np.concatenate([res.results[c]["ctx"] for c in range(N_CORES)], axis=0)
    alpha = np.concatenate([res.results[c]["alpha"] for c in range(N_CORES)], axis=0)
    return context, alpha


# revision 12
# speedup vs baseline: 1.3368x; 1.3368x over previous
"""Bahdanau attention on 8 Trainium2 NeuronCores (Bass/Tile).

Data-parallel over the batch dim: each of the 8 cores gets 32 of the 256
batch rows; all weights are replicated. Per core:

  proj_h = dec @ W_h.T                  (B, H)     [tiny matmul]
  proj_s = X @ W_s.T                    (B, L, H)  [17 GFLOP matmul]
  scores = v . tanh(proj_h + proj_s)    (B, L)
  alpha  = softmax(scores, axis=-1)     (B, L)
  ctx    = alpha @ X                    (B, E)

Device layout: all TensorE-facing data is bf16 (f32 PSUM accumulation).
The host ships X twice — transposed `xt` (e on partitions) for proj_s and
natural `xn` (l on partitions) for ctx — so the device never transposes
the big tensor. proj_s is computed transposed (h on partitions, l on the
free dim), which lets the proj_h add ride the ACT bias operand and makes
the v-dot a K=128, M=1 matmul accumulated over the four h chunks.

The 32 batches run in 4 rounds of 8 (2 col-groups of 4): the M=1 v-dot
and ctx matmuls for 4 batches are packed into distinct PE column groups
via tile_position=(0, 32j) so they stream concurrently, and round
pipelining overlaps the ctx pass's xn DMA with the next round's proj
matmuls.
"""

from contextlib import ExitStack

import ml_dtypes
import numpy as np

import concourse.tile as tile
from concourse import bacc, mybir
from concourse.bass_utils import run_bass_kernel_spmd
from concourse.masks import make_identity

F32 = mybir.dt.float32
BF16 = mybir.dt.bfloat16
AF = mybir.ActivationFunctionType

N_CORES = 8
B_TOTAL, L, E, H = 256, 1024, 512, 512
B = B_TOTAL // N_CORES
RND = 8  # batches per round (softmax batch)
GRP = 4  # batches per PE col-group pack


def build(B=B, L=L, E=E, H=H, reps=1):
    """Build + finalize the per-core Bacc module (SPMD: same on all cores).

    reps>1 wraps the body in a device-side For_i loop — timing only.
    """
    EC, HC, LC, L2 = E // 128, H // 128, L // 128, L // 512
    NR = B // RND
    NG = RND // GRP

    nc = bacc.Bacc()

    xt_d = nc.dram_tensor("xt", [B, EC, 128, L], BF16, kind="ExternalInput")
    xn_d = nc.dram_tensor("xn", [B, LC, 128, E], BF16, kind="ExternalInput")
    wst_d = nc.dram_tensor("wst", [EC, 128, H], BF16, kind="ExternalInput")
    wht_d = nc.dram_tensor("wht", [HC, 128, H], BF16, kind="ExternalInput")
    dect_d = nc.dram_tensor("dect", [HC, 128, B], BF16, kind="ExternalInput")
    vt_d = nc.dram_tensor("vt", [HC, 128, 1], BF16, kind="ExternalInput")
    ctx_d = nc.dram_tensor("ctx", [B, E], F32, kind="ExternalOutput")
    alpha_d = nc.dram_tensor("alpha", [B, L], F32, kind="ExternalOutput")

    with tile.TileContext(nc) as tc, ExitStack() as ctx:
        singles = ctx.enter_context(tc.tile_pool(name="singles", bufs=1))
        xt_pool = ctx.enter_context(tc.tile_pool(name="xt", bufs=GRP + 2))
        xn_pool = ctx.enter_context(tc.tile_pool(name="xn", bufs=GRP + 2))
        tanh_pool = ctx.enter_context(tc.tile_pool(name="tanh", bufs=GRP + 2))
        stage_pool = ctx.enter_context(tc.tile_pool(name="stage", bufs=3))
        rnd_pool = ctx.enter_context(tc.tile_pool(name="rnd", bufs=2))
        pp = ctx.enter_context(tc.tile_pool(name="pp", bufs=2, space="PSUM"))
        ps_sc = ctx.enter_context(tc.tile_pool(name="ps_sc", bufs=3, space="PSUM"))
        ps_mi = ctx.enter_context(tc.tile_pool(name="ps_mi", bufs=1, space="PSUM"))

        # --- static setup: weights, identity, proj_h ---
        wst_sb = singles.tile([128, EC, H], BF16)
        nc.sync.dma_start(out=wst_sb[:], in_=wst_d[:].rearrange("c p h -> p c h"))
        wht_sb = singles.tile([128, HC, H], BF16)
        nc.sync.dma_start(out=wht_sb[:], in_=wht_d[:].rearrange("c p h -> p c h"))
        dect_sb = singles.tile([128, HC, B], BF16)
        nc.sync.dma_start(out=dect_sb[:], in_=dect_d[:].rearrange("c p b -> p c b"))
        v_sb = singles.tile([128, HC, 1], BF16)
        nc.sync.dma_start(out=v_sb[:], in_=vt_d[:].rearrange("c p o -> p c o"))
        ident = singles.tile([RND, RND], F32)
        make_identity(nc, ident[:])

        # proj_h.T in (h, b) layout: ph_sb[p, hc, b] = proj_h[b, hc*128+p]
        ph_sb = singles.tile([128, HC, B], F32)
        for hc in range(HC):
            ph_ps = ps_mi.tile([128, B], F32, tag="mi")
            for kc in range(HC):
                nc.tensor.matmul(
                    ph_ps[:],
                    wht_sb[:, kc, hc * 128 : (hc + 1) * 128],
                    dect_sb[:, kc, :],
                    start=(kc == 0),
                    stop=(kc == HC - 1),
                )
            nc.vector.tensor_copy(ph_sb[:, hc, :], ph_ps[:])

        def emit_pass1(r):
            scores_all = rnd_pool.tile([RND, L], F32, tag="scores_all")

            # ---- pass 1: scores for the round's RND batches ----
            for g in range(NG):
                b0 = r * RND + g * GRP
                xt_ts = []
                for j in range(GRP):
                    xt_t = xt_pool.tile(
                        [128, EC, L], BF16, tag="xt", name=f"xt{b0}_{j}"
                    )
                    nc.sync.dma_start(
                        out=xt_t[:], in_=xt_d[b0 + j].rearrange("c p l -> p c l")
                    )
                    xt_ts.append(xt_t)

                sc_ps = [
                    ps_sc.tile([128, 512], F32, tag="sc", name=f"sc{b0}_{i}")
                    for i in range(L2)
                ]
                for l2 in range(L2):
                    nc.vector.memset(sc_ps[l2][:], 0.0)
                for hc in range(HC):
                    tanh_ts = []
                    for j in range(GRP):
                        proj = pp.tile([128, L], F32, tag="proj", name=f"pj{b0}_{j}")
                        for ec in range(EC):
                            for l2 in range(L2):
                                nc.tensor.matmul(
                                    proj[:, l2 * 512 : (l2 + 1) * 512],
                                    wst_sb[:, ec, hc * 128 : (hc + 1) * 128],
                                    xt_ts[j][:, ec, l2 * 512 : (l2 + 1) * 512],
                                    start=(ec == 0),
                                    stop=(ec == EC - 1),
                                )
                        tanh_t = tanh_pool.tile(
                            [128, L], BF16, tag="tanh", name=f"th{b0}_{j}"
                        )
                        nc.scalar.activation(
                            tanh_t[:], proj[:], AF.Tanh,
                            bias=ph_sb[:, hc, b0 + j : b0 + j + 1],
                        )
                        tanh_ts.append(tanh_t)
                    # 4 batches' v-dots packed into distinct PE col groups
                    for l2 in range(L2):
                        for j in range(GRP):
                            nc.tensor.matmul(
                                sc_ps[l2][32 * j : 32 * j + 1, :],
                                v_sb[:, hc, :],
                                tanh_ts[j][:, l2 * 512 : (l2 + 1) * 512],
                                start=(hc == 0),
                                stop=(hc == HC - 1),
                                tile_position=(0, 32 * j),
                                skip_group_check=True,
                            )
                sc_sb = stage_pool.tile([128, L], F32, tag="sc_sb")
                for l2 in range(L2):
                    nc.vector.tensor_copy(
                        sc_sb[:, l2 * 512 : (l2 + 1) * 512], sc_ps[l2][:]
                    )
                nc.sync.dma_start(
                    out=scores_all[g * GRP : (g + 1) * GRP, :],
                    in_=sc_sb[:].rearrange("(j q) l -> j q l", j=GRP)[:, 0, :],
                )

            return scores_all

        def emit_softmax(r, scores_all):
            # ---- softmax over l (RND rows on partitions) ----
            negmax = rnd_pool.tile([RND, 1], F32, tag="negmax")
            nc.vector.tensor_reduce(
                negmax[:], scores_all[:], axis=mybir.AxisListType.X,
                op=mybir.AluOpType.max, negate=True,
            )
            alpha_sb = rnd_pool.tile([RND, L], F32, tag="alpha")
            denom = rnd_pool.tile([RND, 1], F32, tag="denom")
            nc.scalar.activation(
                alpha_sb[:], scores_all[:], AF.Exp,
                bias=negmax[:], scale=1.0, accum_out=denom[:],
            )
            rden = rnd_pool.tile([RND, 1], F32, tag="rden")
            nc.vector.reciprocal(rden[:], denom[:])
            nc.vector.tensor_scalar_mul(alpha_sb[:], in0=alpha_sb[:], scalar1=rden[:])
            nc.sync.dma_start(out=alpha_d[r * RND : (r + 1) * RND, :], in_=alpha_sb[:])

            return alpha_sb

        def emit_ctx(r, alpha_sb):
            # alpha.T in bf16 for the ctx matmul: alphaT[p, lc, b_local]
            alphaT = rnd_pool.tile([128, LC, RND], BF16, tag="alphaT")
            for lc in range(LC):
                tp = ps_mi.tile([128, RND], F32, tag="mi")
                nc.tensor.transpose(
                    tp[:], alpha_sb[:, lc * 128 : (lc + 1) * 128], ident[:]
                )
                nc.vector.tensor_copy(alphaT[:, lc, :], tp[:])

            # ---- pass 2: ctx = alpha @ X (4 batches per PE col-group pack) ----
            for g in range(NG):
                b0 = r * RND + g * GRP
                xn_ts = []
                for j in range(GRP):
                    xn_t = xn_pool.tile(
                        [128, LC, E], BF16, tag="xn", name=f"xn{b0}_{j}"
                    )
                    nc.sync.dma_start(
                        out=xn_t[:], in_=xn_d[b0 + j].rearrange("c p e -> p c e")
                    )
                    xn_ts.append(xn_t)
                ctx_ps = ps_mi.tile([128, E], F32, tag="mi", name=f"cp{b0}")
                nc.vector.memset(ctx_ps[:], 0.0)
                for lc in range(LC):
                    for j in range(GRP):
                        nc.tensor.matmul(
                            ctx_ps[32 * j : 32 * j + 1, :],
                            alphaT[:, lc, g * GRP + j : g * GRP + j + 1],
                            xn_ts[j][:, lc, :],
                            start=(lc == 0),
                            stop=(lc == LC - 1),
                            tile_position=(0, 32 * j),
                            skip_group_check=True,
                        )
                ctx_sb = stage_pool.tile([128, E], F32, tag="ctx_sb")
                nc.vector.tensor_copy(ctx_sb[:], ctx_ps[:])
                nc.sync.dma_start(
                    out=ctx_d[b0 : b0 + GRP, :],
                    in_=ctx_sb[:].rearrange("(j q) e -> j q e", j=GRP)[:, 0, :],
                )

        def emit_body():
            pend = None
            for r in range(B // RND):
                if pend is not None:
                    al = emit_softmax(pend[0], pend[1])
                    pend = (pend[0], pend[1], al)
                sc = emit_pass1(r)
                if pend is not None:
                    emit_ctx(pend[0], pend[2])
                pend = (r, sc)
            al = emit_softmax(pend[0], pend[1])
            emit_ctx(pend[0], al)

        if reps == 1:
            emit_body()
        else:
            with tc.For_i(0, reps):
                emit_body()

    nc.finalize()
    return nc


def host_prep(decoder_hidden, encoder_outputs, W_h, W_s, v, n_cores=N_CORES):
    """Full f32 inputs -> per-core input dicts in the device bf16 layouts."""
    bf16 = ml_dtypes.bfloat16
    BT, L_, E_ = encoder_outputs.shape
    H_ = W_h.shape[0]
    Bc = BT // n_cores

    x16 = np.ascontiguousarray(encoder_outputs).astype(bf16)
    # xt[b, ec, p, l] = X[b, l, ec*128+p]
    xt = np.ascontiguousarray(x16.reshape(BT, L_, E_ // 128, 128).transpose(0, 2, 3, 1))
    # xn[b, lc, p, e] = X[b, lc*128+p, e]  (pure view)
    xn = x16.reshape(BT, L_ // 128, 128, E_)
    # wst[ec, p, h] = W_s[h, ec*128+p]
    wst = np.ascontiguousarray(W_s.astype(bf16).reshape(H_, E_ // 128, 128).transpose(1, 2, 0))
    wht = np.ascontiguousarray(W_h.astype(bf16).reshape(H_, H_ // 128, 128).transpose(1, 2, 0))
    # dect[kc, p, b] = dec[b, kc*128+p]
    dect_full = np.ascontiguousarray(
        decoder_hidden.astype(bf16).reshape(BT, H_ // 128, 128).transpose(1, 2, 0)
    )
    vt = np.ascontiguousarray(v.astype(bf16).reshape(H_ // 128, 128, 1))

    in_maps = []
    for c in range(n_cores):
        sl = slice(c * Bc, (c + 1) * Bc)
        in_maps.append(
            {
                "xt": xt[sl],
                "xn": xn[sl],
                "wst": wst,
                "wht": wht,
                "dect": np.ascontiguousarray(dect_full[:, :, sl]),
                "vt": vt,
            }
        )
    return in_maps


_NC_CACHE = {}


def _get_nc(reps=1):
    if reps not in _NC_CACHE:
        _NC_CACHE[reps] = build(reps=reps)
    return _NC_CACHE[reps]


def kernel(decoder_hidden, encoder_outputs, W_h, W_s, v):
    decoder_hidden = np.asarray(decoder_hidden, dtype=np.float32)
    encoder_outputs = np.asarray(encoder_outputs, dtype=np.float32)
    W_h = np.asarray(W_h, dtype=np.float32)
    W_s = np.asarray(W_s, dtype=np.float32)
    v = np.asarray(v, dtype=np.float32)
    assert decoder_hidden.shape == (B_TOTAL, H)
    assert encoder_outputs.shape == (B_TOTAL, L, E)

    in_maps = host_prep(decoder_hidden, encoder_outputs, W_h, W_s, v)
    nc = _get_nc(reps=1)
    res = run_bass_kernel_spmd(nc, in_maps, core_ids=list(range(N_CORES)))
    context = np.concatenate([res.results[c]["ctx"] for c in range(N_CORES)], axis=0)
    alpha = np.concatenate([res.results[c]["alpha"] for c in range(N_CORES)], axis=0)
    return context, alpha
